# revision 1
# baseline (speedup 1.0000x reference)
"""Trainium2 Bass kernel: conditional logistic regression (segmented softmax).

Problem: X [N=4194304, 64] fp32, sorted segment_ids [N] (65536 segments,
avg 64 rows), W [1,64], b [1].
  logits = X @ W.T + b ; out = segmented_softmax(logits, segment_ids)

Strategy (8 cores, data-parallel over N):
  - Each core owns N/8 = 524288 consecutive rows, split into 128 spans of
    S = 4096 rows (one span per SBUF partition).
  - Overlap-pad trick: each partition processes a window [span_start - PAD,
    span_end + PAD) where PAD >= max segment length, so every segment whose
    rows intersect the core rectangle is fully contained in the window.
    Rows outside the window's core rectangle are computed redundantly and
    discarded; no cross-partition/cross-core communication is needed.
  - X is shipped as fp16 (halves HBM traffic; products are accumulated in
    fp32 so the logit error stays ~5e-4). Segment-boundary masks (keep /
    not-end) are precomputed on host as fp16 0/1 arrays.
  - Matvec split across engines by per-chunk quotas:
    * pe: natural-layout tile is PE-transposed (PSUM fp16), batch-copied to
      SBUF by ACT (4 tiles per copy), then matmul'd against W2 [128, 2].
    * dve/gp: elementwise multiply by a materialized [128, CF] W tile (flat
      stride-1 fp16 keeps DVE in 2x perf mode), then DVE fp16 reduce.
  - exp on ACT (b dropped: constant shift cancels in softmax).
  - Segmented sums via DVE tensor_tensor_scan: forward masked-sum scan
    (reset at segment starts), then a backward propagate scan spreads each
    segment's total back over its rows. out = E * recip(denom), written fp16.
"""

import numpy as np

import concourse.bass as bass
import concourse.tile as tile
from concourse import mybir
from concourse.alu_op_type import AluOpType

F32 = mybir.dt.float32
F16 = mybir.dt.float16

# Full problem constants
N_FULL = 4194304
D = 64
N_CORES = 8
SPANS = 128


def _rev(ap):
    """Reverse an AP along its (last) free dim."""
    return ap[:, ::-1]


def _split_multi_waits(nc):
    """Hoist extra sync waits into standalone EventSemaphore instructions.

    Engine compute/DMA instruction encodings only support a single sync-wait
    slot (walrus: "Too many sync wait commands"); standalone waits execute on
    the same engine sequencer in program order, so semantics are unchanged.
    """
    exempt = ()
    n = 0
    for f in nc.m.functions:
        for blk in f.blocks:
            insts = list(blk.instructions)
            out = []
            for ins in insts:
                si = ins.sync_info
                if (
                    si is not None
                    and si.on_wait
                    and len(si.on_wait) > 1
                    and type(ins).__name__ not in exempt
                ):
                    waits = list(si.on_wait)
                    for w in waits[:-1]:
                        es = mybir.InstEventSemaphore(
                            name=f"W-split-{n}", ins=[], outs=[]
                        )
                        n += 1
                        es.engine = ins.engine
                        es.sync_info = mybir.SyncInfo(on_wait=[w], on_update=[])
                        nc.inst_map[es.name] = es
                        out.append(es)
                    ins.sync_info = mybir.SyncInfo(
                        on_wait=[waits[-1]], on_update=list(si.on_update)
                    )
                out.append(ins)
            if len(out) != len(insts):
                blk.instructions = out
    return n


# per-chunk path quotas tuned from traces; gp folds the d-halves on GPSIMD
# before the DVE reduce, gpn leaves the full-width reduce to DVE
QUOTAS = {"pe": 36, "dve": 12, "gp": 10, "gpn": 10}


def _make_paths(nchunks, quotas=QUOTAS):
    total = sum(quotas.values())
    acc = {k: 0.0 for k in quotas}
    paths = []
    for _ in range(nchunks):
        for k in quotas:
            acc[k] += quotas[k] / total
        k = max(acc, key=lambda q: acc[q])
        acc[k] -= 1.0
        paths.append(k)
    return paths


def build_nc(S, PAD, CF=4096, quotas=QUOTAS, trn=None):
    L = S + 2 * PAD
    rows_per_chunk = CF // D
    assert (L * D) % CF == 0
    nchunks = (L * D) // CF
    tiles_per_chunk = CF // 128
    assert tiles_per_chunk * 2 == rows_per_chunk
    assert tiles_per_chunk % 4 == 0
    L_tot = SPANS * S + 2 * PAD
    paths = _make_paths(nchunks, quotas)

    nc = bass.Bass(trn, target_bir_lowering=False)
    xs = nc.dram_tensor("xs", [L_tot * D], F16, kind="ExternalInput")
    keepg = nc.dram_tensor("keepg", [L_tot + 1], F32, kind="ExternalInput")
    nendg = nc.dram_tensor("nendg", [L_tot], F32, kind="ExternalInput")
    w2 = nc.dram_tensor("w2", [128, 2], F16, kind="ExternalInput")
    wrep = nc.dram_tensor("wrep", [1, 1024], F16, kind="ExternalInput")
    ident = nc.dram_tensor("ident", [128, 128], F16, kind="ExternalInput")
    out = nc.dram_tensor("out", [SPANS * S], F16, kind="ExternalOutput")

    with tile.TileContext(nc) as tc:
        with (
            tc.tile_pool(name="xin", bufs=10) as xin_pool,
            tc.tile_pool(name="tsb", bufs=4) as tsb_pool,
            tc.tile_pool(name="lg", bufs=4) as lg_pool,
            tc.tile_pool(name="tps", bufs=4, space="PSUM") as tpsum_pool,
            tc.tile_pool(name="pps", bufs=2, space="PSUM") as ppsum_pool,
            tc.tile_pool(name="big", bufs=1) as big,
        ):
            w2_sb = big.tile([128, 2], F16, tag="w2")
            nc.sync.dma_start(out=w2_sb[:, :], in_=w2[:, :])
            id_sb = big.tile([128, 128], F16, tag="ident")
            nc.sync.dma_start(out=id_sb[:, :], in_=ident[:, :])
            # W tiled out to a full [128, CF] tile: flat stride-1 fp16
            # operands keep the DVE multiply in the 2x 16-bit perf mode
            wb_sb = big.tile([128, CF], F16, tag="wb")
            wv_ap = wrep[:, :]
            nc.sync.dma_start(
                out=wb_sb[:, :],
                in_=bass.AP(tensor=wv_ap.tensor, offset=wv_ap.offset,
                            ap=[[0, 128], [0, CF // 1024], [1, 1024]]),
            )
            # absorb const-DMA ticks into PE's clock (LDW has 1 wait slot)
            with tc.tile_pool(name="dummy", bufs=1, space="PSUM") as dpool:
                dtile = dpool.tile([128, 128], F16, tag="d")
                nc.tensor.transpose(dtile[:, :], id_sb[:, :], id_sb[:, :])
                dtf = dpool.tile([128, 2], F32, tag="df")
                nc.tensor.matmul(dtf[:, :], lhsT=id_sb[:, :], rhs=w2_sb[:, :],
                                 start=True, stop=True)

            keep = big.tile([SPANS, L + 1], F32, tag="keep")
            nc.sync.dma_start(
                out=keep[:, :],
                in_=bass.AP(tensor=keepg, offset=0, ap=[[S, SPANS], [1, L + 1]]),
            )
            nc.vector.memset(keep[:, 0:1], 0.0)
            nc.vector.memset(keep[:, L : L + 1], 0.0)
            nend = big.tile([SPANS, L], F32, tag="nend")
            nc.scalar.dma_start(
                out=nend[:, :],
                in_=bass.AP(tensor=nendg, offset=0, ap=[[S, SPANS], [1, L]]),
            )

            E = big.tile([SPANS, L], F32, tag="E")
            s_run = big.tile([SPANS, L], F32, tag="srun")
            evh_pool = ctx_evh = tc.tile_pool(name="evh", bufs=2)
            evh_pool = ctx_evh.__enter__()

            # Sub-window softmax pipelines: split the core [PAD, PAD+S) into
            # H parts; each part's segments live within its part +- PAD, so
            # each sub-window [a,b) runs its own scan chain as soon as its E
            # columns exist (overlap-pad trick applied recursively).
            H = 4
            Sh = S // H
            sub = []
            for h in range(H):
                a = h * Sh
                b = min(L, PAD + (h + 1) * Sh + PAD)
                done_chunk = -(-b // rows_per_chunk) - 1
                sub.append((a, b, done_chunk))

            def emit_subwindow(h):
                a, b, _ = sub[h]
                w = b - a
                assert w <= 4095
                nc.vector.tensor_tensor_scan(
                    out=s_run[:, a:b], data0=keep[:, a:b], data1=E[:, a:b],
                    initial=0.0, op0=AluOpType.mult, op1=AluOpType.add,
                )
                evh = evh_pool.tile([SPANS, w], F32, tag="evh")
                nc.vector.tensor_tensor(
                    out=evh[:, :], in0=s_run[:, a:b], in1=nend[:, a:b],
                    op=AluOpType.mult,
                )
                nc.vector.tensor_tensor_scan(
                    out=_rev(s_run[:, a:b]), data0=_rev(keep[:, a + 1 : b + 1]),
                    data1=_rev(evh[:, :]), initial=0.0,
                    op0=AluOpType.mult, op1=AluOpType.add,
                )
                c0, c1 = PAD + h * Sh, PAD + (h + 1) * Sh
                core = s_run[:, c0:c1]
                # 1/denom as exp(-ln(denom)) on ACT: denom is a positive sum
                # of exps (core rows always hold a full segment total), so ln
                # is safe; keeps the iterative reciprocal off the busy DVE
                nc.scalar.activation(
                    out=core, in_=core, func=mybir.ActivationFunctionType.Ln,
                )
                nc.scalar.activation(
                    out=core, in_=core, func=mybir.ActivationFunctionType.Exp,
                    scale=-1.0,
                )
                ot = evh_pool.tile([SPANS, Sh], F16, tag="ot")
                nc.vector.tensor_tensor(
                    out=ot[:, :], in0=E[:, c0:c1], in1=core, op=AluOpType.mult,
                )
                dma = nc.sync if h % 2 == 0 else nc.scalar
                dma.dma_start(
                    out=bass.AP(tensor=out, offset=h * Sh,
                                ap=[[S, SPANS], [1, Sh]]),
                    in_=ot[:, :],
                )

            # Software pipeline with explicit stage lags. Every engine runs
            # its instruction stream in order, so a consumer emitted right
            # after its cross-engine producer stalls that engine's whole
            # queue (head-of-line). Lagging each stage by whole chunks keeps
            # producers comfortably ahead of consumers.
            LAG_MM = 2   # matmul groups behind transposes/copies (PE stream)
            LAG_RED = 2  # DVE reduce behind its chunk's mult/fold
            LAG_EXP = 3  # ACT exp behind its chunk
            pend_mm = []   # (chunk, emit-4-matmuls closure)
            pend_red = []  # (chunk, closure)
            pend_exp = []  # (chunk, closure)
            sub_emitted = set()
            lg_by_chunk = {}

            def drain_mm(keep, upto_chunk=None):
                while len(pend_mm) > keep or (
                    pend_mm and upto_chunk is not None
                    and pend_mm[0][0] <= upto_chunk
                ):
                    pend_mm.pop(0)[1]()

            def drain(j):
                while pend_red and pend_red[0][0] <= j - LAG_RED:
                    pend_red.pop(0)[1]()
                while pend_exp and pend_exp[0][0] <= j - LAG_EXP:
                    jj, fn = pend_exp.pop(0)
                    drain_mm(0, upto_chunk=jj)
                    fn()
                    for h in range(H):
                        if sub[h][2] == jj and h not in sub_emitted:
                            sub_emitted.add(h)
                            emit_subwindow(h)

            for j in range(nchunks):
                xc = xin_pool.tile([SPANS, CF], F16, tag="xc")
                # all X loads on the SP ring: the ACT ring's trigger slots sit
                # behind copies that transitively wait on these loads, which
                # would cap DMA run-ahead at ~1 chunk
                dma_eng = nc.sync
                dma_eng.dma_start(
                    out=xc[:, :],
                    in_=bass.AP(
                        tensor=xs, offset=j * CF, ap=[[S * D, SPANS], [1, CF]]
                    ),
                )
                drain(j)
                e_slice = E[:, j * rows_per_chunk : (j + 1) * rows_per_chunk]
                if paths[j] == "pe":
                    P = ppsum_pool.tile([SPANS, rows_per_chunk], F32, tag="P")
                    for t4 in range(tiles_per_chunk // 4):
                        tr4 = tpsum_pool.tile([128, 512], F16, tag="tr")
                        for q in range(4):
                            t = t4 * 4 + q
                            nc.tensor.transpose(
                                tr4[:, q * 128 : (q + 1) * 128],
                                xc[:, t * 128 : (t + 1) * 128], id_sb[:, :],
                            )
                        tsb = tsb_pool.tile([128, 512], F16, tag="tsb")
                        nc.scalar.copy(out=tsb[:, :], in_=tr4[:, :])

                        def mm_group(P=P, tsb=tsb, t4=t4):
                            for q in range(4):
                                t = t4 * 4 + q
                                nc.tensor.matmul(
                                    P[:, 2 * t : 2 * t + 2],
                                    lhsT=tsb[:, q * 128 : (q + 1) * 128],
                                    rhs=w2_sb[:, :], start=True, stop=True,
                                )
                        pend_mm.append((j, mm_group))
                        drain_mm(LAG_MM)

                    def pe_exp(P=P, e_slice=e_slice):
                        nc.scalar.activation(
                            out=e_slice, in_=P[:, :],
                            func=mybir.ActivationFunctionType.Exp,
                        )
                    pend_exp.append((j, pe_exp))
                else:
                    eng = nc.vector if paths[j] == "dve" else nc.gpsimd
                    eng.tensor_tensor(
                        out=xc[:, :], in0=xc[:, :], in1=wb_sb[:, :],
                        op=AluOpType.mult,
                    )
                    xc3 = xc[:, :].rearrange("p (r d) -> p r d", d=D)
                    red_w = D
                    if paths[j] == "gp":
                        # GPSIMD folds the top d-half into the bottom so the
                        # DVE reduce only reads half the elements
                        nc.gpsimd.tensor_tensor(
                            out=xc3[:, :, 0 : D // 2], in0=xc3[:, :, 0 : D // 2],
                            in1=xc3[:, :, D // 2 : D], op=AluOpType.add,
                        )
                        red_w = D // 2

                    def red_tail(xc3=xc3, j=j, red_w=red_w):
                        lg = lg_pool.tile([SPANS, rows_per_chunk], F16, tag="lg")
                        with nc.allow_low_precision(
                            "fp16 logit write; DVE reduce state stays fp32"
                        ):
                            nc.vector.tensor_reduce(
                                out=lg[:, :], in_=xc3[:, :, 0:red_w],
                                axis=mybir.AxisListType.X, op=AluOpType.add,
                            )
                        lg_by_chunk[j] = lg
                    pend_red.append((j, red_tail))

                    def exp_tail(j=j, e_slice=e_slice):
                        nc.scalar.activation(
                            out=e_slice, in_=lg_by_chunk.pop(j)[:, :],
                            func=mybir.ActivationFunctionType.Exp,
                        )
                    pend_exp.append((j, exp_tail))
            drain(nchunks - 1 + max(LAG_RED, LAG_EXP))
            drain_mm(0)
            assert not pend_red and not pend_exp and len(sub_emitted) == H

            ctx_evh.__exit__(None, None, None)
    _split_multi_waits(nc)
    return nc


def _prep_host(X, segment_ids, W, S, PAD, n_cores):
    N = X.shape[0]
    n_c = SPANS * S
    assert n_c * n_cores == N

    ids = np.asarray(segment_ids).astype(np.int64)
    idsp = np.concatenate(
        [np.full(PAD, -1, np.int64), ids, np.full(PAD + 1, -1, np.int64)]
    )
    eq = idsp[1:] == idsp[:-1]
    keep_g = np.concatenate([[False], eq]).astype(np.float32)
    nend_g = (~eq).astype(np.float32)

    Xf = np.asarray(X, dtype=np.float32).astype(np.float16)
    Xp = np.concatenate(
        [np.zeros((PAD, D), np.float16), Xf, np.zeros((PAD, D), np.float16)]
    )
    Wf = np.asarray(W, np.float32).reshape(-1).astype(np.float16)
    w2 = np.zeros((128, 2), np.float16)
    w2[0:64, 0] = Wf
    w2[64:128, 1] = Wf
    ident = np.eye(128, dtype=np.float16)
    in_maps = []
    for k in range(n_cores):
        lo = k * n_c
        hi = (k + 1) * n_c + 2 * PAD
        in_maps.append(
            {
                "xs": np.ascontiguousarray(Xp[lo:hi]).reshape(-1),
                "keepg": np.ascontiguousarray(keep_g[lo : hi + 1]),
                "nendg": np.ascontiguousarray(nend_g[lo:hi]),
                "w2": w2,
                "wrep": np.tile(Wf, 16).reshape(1, 1024),
                "ident": ident,
            }
        )
    return in_maps


def kernel(X, segment_ids, W, b, _return_results=False, _trace=False):
    from concourse import bass_utils

    X = np.asarray(X)
    N = X.shape[0]
    assert N == N_FULL, f"kernel hardcoded for N={N_FULL}, got {N}"
    S = N // (N_CORES * SPANS)
    m = _max_seg_len(segment_ids)
    PAD = max(64, int(np.ceil(m / 64.0)) * 64)

    nc = build_nc(S, PAD)
    in_maps = _prep_host(X, segment_ids, W, S, PAD, N_CORES)
    res = bass_utils.run_bass_kernel_spmd(
        nc, in_maps, core_ids=list(range(N_CORES)), trace=_trace
    )
    out = np.concatenate(
        [np.asarray(r["out"], dtype=np.float32) for r in res.results]
    )
    if _return_results:
        return out, res
    return out


def _max_seg_len(segment_ids):
    ids = np.asarray(segment_ids).astype(np.int64)
    change = np.flatnonzero(np.diff(ids) != 0)
    starts = np.concatenate([[0], change + 1])
    ends = np.concatenate([change + 1, [len(ids)]])
    return int((ends - starts).max())



# revision 2
# speedup vs baseline: 1.6556x; 1.6556x over previous
"""Trainium2 Bass kernel: conditional logistic regression (segmented softmax).

Problem: X [N=4194304, 64] fp32, sorted segment_ids [N] (65536 segments,
avg 64 rows), W [1,64], b [1].
  logits = X @ W.T + b ; out = segmented_softmax(logits, segment_ids)

Strategy (8 cores, data-parallel over N), v2 "PE-matvec" design:
  - Each core owns N/8 = 524288 consecutive rows as 128 spans of S = 4096
    rows (one span per SBUF partition), with +-PAD overlap per span so
    every segment intersecting a span's core rows is fully inside its
    window (PAD >= max segment length). Rows in the pad are computed
    redundantly and discarded; no cross-partition communication.
  - The matvec runs entirely on PE. The host pre-packs X (fp16) into
    "pair-transposed" tiles: rhs[k = p*64 + d, f] = X[row(2c+p, g, f), d],
    so one matmul against a [128, 2]-shaped W (packed twice along the
    128-contraction) yields logits for two spans at a time. A sliding
    window into a [128, 2*SPANS] zero tile places span-pair c's logits at
    PSUM partitions (2c, 2c+1); 64 accumulating matmuls build a full
    [128, F] PSUM tile of logits laid out [span, position] - exactly the
    layout the segmented-softmax scans need. No on-chip transposes, no
    PSUM->SBUF copies, no DVE/GPSIMD multiplies for the matvec.
  - ACT applies exp directly PSUM->SBUF (b dropped: constant shift
    cancels in softmax).
  - Segmented sums via DVE tensor_tensor_scan as in v1: forward masked-
    sum scan (reset at segment starts), backward propagate scan spreads
    each segment total over its rows; out = E * exp(-ln(denom)), fp16.
    Sub-windows (H=5, staggered against PSUM group completion) start
    scanning as soon as their E columns exist.
  - Segment-boundary masks (keep / not-end) are fp16 0/1 host arrays
    (scan state is fp32 regardless of operand dtype).
"""

import numpy as np

import concourse.bass as bass
import concourse.tile as tile
from concourse import mybir
from concourse.alu_op_type import AluOpType

F32 = mybir.dt.float32
F16 = mybir.dt.float16

# Full problem constants
N_FULL = 4194304
D = 64
N_CORES = 8
SPANS = 128
F_COL = 448  # PSUM group width (<= 512 fp32 = one PSUM bank)


def _rev(ap):
    """Reverse an AP along its (last) free dim."""
    return ap[:, ::-1]


def _split_multi_waits(nc):
    """Hoist extra sync waits into standalone EventSemaphore instructions.

    Engine compute/DMA instruction encodings only support a single sync-wait
    slot (walrus: "Too many sync wait commands"); standalone waits execute on
    the same engine sequencer in program order, so semantics are unchanged.
    """
    exempt = ()
    n = 0
    for f in nc.m.functions:
        for blk in f.blocks:
            insts = list(blk.instructions)
            out = []
            for ins in insts:
                si = ins.sync_info
                if (
                    si is not None
                    and si.on_wait
                    and len(si.on_wait) > 1
                    and type(ins).__name__ not in exempt
                ):
                    waits = list(si.on_wait)
                    for w in waits[:-1]:
                        es = mybir.InstEventSemaphore(
                            name=f"W-split-{n}", ins=[], outs=[]
                        )
                        n += 1
                        es.engine = ins.engine
                        es.sync_info = mybir.SyncInfo(on_wait=[w], on_update=[])
                        nc.inst_map[es.name] = es
                        out.append(es)
                    ins.sync_info = mybir.SyncInfo(
                        on_wait=[waits[-1]], on_update=list(si.on_update)
                    )
                out.append(ins)
            if len(out) != len(insts):
                blk.instructions = out
    return n


def _make_splits(S, PAD, F, NG):
    """Core-row split points for the scan sub-windows, staggered so each
    window's E columns complete one PSUM group apart near the tail."""
    targets = sorted({max(0, NG - 7), max(0, NG - 5), max(0, NG - 3),
                      max(0, NG - 2)})
    splits = []
    prev = 0
    for g in targets:
        e = F * (g + 1) - 2 * PAD
        if e <= prev + 256 or e >= S - 256:
            continue
        splits.append(e)
        prev = e
    splits.append(S)
    return splits


def build_nc(S, PAD, F=F_COL, spans=SPANS, splits=None, trn=None):
    L = S + 2 * PAD
    assert L % F == 0
    NG = L // F
    MMG = spans // 2          # matmuls (span pairs) per PSUM group
    HALF = MMG // 2           # matmuls per DMA chunk
    L_tot = spans * S + 2 * PAD
    if splits is None:
        splits = _make_splits(S, PAD, F, NG)
    assert splits[-1] == S

    nc = bass.Bass(trn, target_bir_lowering=False)
    xs = nc.dram_tensor("xs", [NG * 2 * 128 * HALF * F], F16,
                        kind="ExternalInput")
    keepg = nc.dram_tensor("keepg", [L_tot + 1], F16, kind="ExternalInput")
    nendg = nc.dram_tensor("nendg", [L_tot], F16, kind="ExternalInput")
    zz = nc.dram_tensor("zz", [128, 2 * spans], F16, kind="ExternalInput")
    out = nc.dram_tensor("out", [spans * S], F16, kind="ExternalOutput")

    with tile.TileContext(nc) as tc:
        with (
            tc.tile_pool(name="xin", bufs=3) as xin_pool,
            tc.tile_pool(name="pps", bufs=2, space="PSUM") as ppsum_pool,
            tc.tile_pool(name="evh", bufs=2) as evh_pool,
            tc.tile_pool(name="big", bufs=1) as big,
        ):
            zz_sb = big.tile([128, 2 * spans], F16, tag="zz")
            nc.scalar.dma_start(out=zz_sb[:, :], in_=zz[:, :])

            keep = big.tile([spans, L + 1], F16, tag="keep")
            nc.scalar.dma_start(
                out=keep[:, :],
                in_=bass.AP(tensor=keepg, offset=0, ap=[[S, spans], [1, L + 1]]),
            )
            nc.vector.memset(keep[:, 0:1], 0.0)
            nc.vector.memset(keep[:, L : L + 1], 0.0)
            nend = big.tile([spans, L], F16, tag="nend")
            nc.scalar.dma_start(
                out=nend[:, :],
                in_=bass.AP(tensor=nendg, offset=0, ap=[[S, spans], [1, L]]),
            )

            E = big.tile([spans, L], F32, tag="E")
            s_run = big.tile([spans, L], F32, tag="srun")

            # sub-windows: window h covers core rows [e_{h-1}, e_h), scans
            # over [e_{h-1}, e_h + 2*PAD) in L-coords; ready once the PSUM
            # group covering column b-1 has been exp'd.
            sub = []
            prev = 0
            for e in splits:
                b = min(L, e + 2 * PAD)
                ready = -(-b // F) - 1
                sub.append((prev, e, b, ready))
                prev = e

            def emit_subwindow(h):
                a, e, b, _ = sub[h]
                w = b - a
                assert w <= 4095
                nc.vector.tensor_tensor_scan(
                    out=s_run[:, a:b], data0=keep[:, a:b], data1=E[:, a:b],
                    initial=0.0, op0=AluOpType.mult, op1=AluOpType.add,
                )
                evh = evh_pool.tile([spans, w], F32, tag="evh")
                nc.vector.tensor_tensor(
                    out=evh[:, :], in0=s_run[:, a:b], in1=nend[:, a:b],
                    op=AluOpType.mult,
                )
                nc.vector.tensor_tensor_scan(
                    out=_rev(s_run[:, a:b]), data0=_rev(keep[:, a + 1 : b + 1]),
                    data1=_rev(evh[:, :]), initial=0.0,
                    op0=AluOpType.mult, op1=AluOpType.add,
                )
                c0, c1 = PAD + a, PAD + e
                core = s_run[:, c0:c1]
                # 1/denom as exp(-ln(denom)) on ACT: denom is a positive sum
                # of exps (core rows always hold a full segment total)
                nc.scalar.activation(
                    out=core, in_=core, func=mybir.ActivationFunctionType.Ln,
                )
                nc.scalar.activation(
                    out=core, in_=core, func=mybir.ActivationFunctionType.Exp,
                    scale=-1.0,
                )
                ot = evh_pool.tile([spans, e - a], F16, tag="ot")
                nc.vector.tensor_tensor(
                    out=ot[:, :], in0=E[:, c0:c1], in1=core, op=AluOpType.mult,
                )
                nc.gpsimd.dma_start(
                    out=bass.AP(tensor=out, offset=a,
                                ap=[[S, spans], [1, e - a]]),
                    in_=ot[:, :],
                )

            emitted = set()
            for g in range(NG):
                P = ppsum_pool.tile([spans, F], F32, tag="P")
                for half in range(2):
                    xh = xin_pool.tile([128, HALF * F], F16, tag="xh")
                    nc.sync.dma_start(
                        out=xh[:, :],
                        in_=bass.AP(
                            tensor=xs,
                            offset=(g * 2 + half) * 128 * HALF * F,
                            ap=[[HALF * F, 128], [1, HALF * F]],
                        ),
                    )
                    for cc in range(HALF):
                        c = half * HALF + cc
                        nc.tensor.matmul(
                            P[:, :],
                            lhsT=zz_sb[:, spans - 2 * c : 2 * spans - 2 * c],
                            rhs=xh[:, cc * F : (cc + 1) * F],
                            start=(c == 0), stop=(c == MMG - 1),
                        )
                nc.scalar.activation(
                    out=E[:, g * F : (g + 1) * F], in_=P[:, :],
                    func=mybir.ActivationFunctionType.Exp,
                )
                for h in range(len(sub)):
                    if sub[h][3] == g and h not in emitted:
                        emitted.add(h)
                        emit_subwindow(h)
            assert len(emitted) == len(sub), (emitted, sub)
    _split_multi_waits(nc)
    return nc


def _prep_host(X, segment_ids, W, S, PAD, F=F_COL, spans=SPANS, n_cores=N_CORES):
    N = X.shape[0]
    n_c = spans * S
    assert n_c * n_cores == N
    L = S + 2 * PAD
    NG = L // F
    MMG = spans // 2
    HALF = MMG // 2

    ids = np.asarray(segment_ids).astype(np.int64)
    idsp = np.concatenate(
        [np.full(PAD, -1, np.int64), ids, np.full(PAD + 1, -1, np.int64)]
    )
    eq = idsp[1:] == idsp[:-1]
    keep_g = np.concatenate([[False], eq]).astype(np.float16)
    nend_g = (~eq).astype(np.float16)

    Xf = np.asarray(X, dtype=np.float32).astype(np.float16)
    Xp = np.concatenate(
        [np.zeros((PAD, D), np.float16), Xf, np.zeros((PAD, D), np.float16)]
    )
    Wf = np.asarray(W, np.float32).reshape(-1).astype(np.float16)
    zz = np.zeros((128, 2 * spans), np.float16)
    zz[0:64, spans] = Wf
    zz[64:128, spans + 1] = Wf

    st = Xp.strides
    in_maps = []
    for k in range(n_cores):
        lo = k * n_c
        Xc = Xp[lo : lo + n_c + 2 * PAD]
        # V[q, i, d] = row (q*S + i - PAD) of this core's slice
        V = np.lib.stride_tricks.as_strided(
            Xc, shape=(spans, L, D), strides=(S * st[0], st[0], st[1])
        )
        # [c, p, i, d] -> [half, cc, p, g, f, d] -> [g, half, p, d, cc, f]
        V6 = V.reshape(2, HALF, 2, NG, F, D)
        xs = np.ascontiguousarray(V6.transpose(3, 0, 2, 5, 1, 4)).reshape(-1)
        in_maps.append(
            {
                "xs": xs,
                "keepg": np.ascontiguousarray(keep_g[lo : lo + n_c + 2 * PAD + 1]),
                "nendg": np.ascontiguousarray(nend_g[lo : lo + n_c + 2 * PAD]),
                "zz": zz,
            }
        )
    return in_maps


def _max_seg_len(segment_ids):
    ids = np.asarray(segment_ids).astype(np.int64)
    change = np.flatnonzero(np.diff(ids) != 0)
    starts = np.concatenate([[0], change + 1])
    ends = np.concatenate([change + 1, [len(ids)]])
    return int((ends - starts).max())


def _choose_pad(S, m, F=F_COL):
    """Smallest PAD >= max(128, m) with (S + 2*PAD) % F == 0."""
    p = max(128, m)
    while (S + 2 * p) % F != 0:
        p += 1
    return p


def kernel(X, segment_ids, W, b, _return_results=False, _trace=False):
    from concourse import bass_utils

    X = np.asarray(X)
    N = X.shape[0]
    assert N == N_FULL, f"kernel hardcoded for N={N_FULL}, got {N}"
    S = N // (N_CORES * SPANS)
    m = _max_seg_len(segment_ids)
    PAD = _choose_pad(S, m)

    nc = build_nc(S, PAD)
    in_maps = _prep_host(X, segment_ids, W, S, PAD)
    res = bass_utils.run_bass_kernel_spmd(
        nc, in_maps, core_ids=list(range(N_CORES)), trace=_trace
    )
    out = np.concatenate(
        [np.asarray(r["out"], dtype=np.float32) for r in res.results]
    )
    if _return_results:
        return out, res
    return out
